# revision 1
# baseline (speedup 1.0000x reference)
"""GRU decoder kernel for Trainium2 (8 NeuronCores, data-parallel over batch).

Math (PyTorch GRU, gate order r,z,n), per batch element:
    gx_t = x_t * w_ih + b_ih              (input dim == 1 -> rank-1)
    gh_t = h_{t-1} @ w_hh.T + b_hh
    r = sigmoid(gx_r + gh_r); z = sigmoid(gx_z + gh_z)
    n = tanh(gx_n + b_ih_n + r * (gh_n + b_hh_n))
    h_t = (1-z)*n + z*h_{t-1}
    out = h_T @ fc_w.T + fc_b

Device layout (per core, B_c = 1024 batch):
  - H [128, 512] f16: partitions 0-63 = hidden coords for batch 0-511 (u),
    partitions 64-127 = hidden for batch 512-1023 (v); free dim = batch.
  - Two phase-shifted batch groups (free-dim halves of 256) pipeline the
    per-step chain; 2 PSUM banks per group, double-buffered = all 8 banks.
  - Per group-step, PSUM bank A holds [R | Z] pre-activations (free 0:256 =
    r, 256:512 = z), bank B holds [NH | NX].  One sigmoid ACT op covers both
    r and z; biases b_r/b_z ride into PSUM through a ones-row in the X tile
    (blocks of 63 timesteps + 1 ones row), multiplied by a bias row in the
    one-hot lhsT.
  - Matmuls use all four 64x64 PE quadrants: h-matmuls on row-quadrant of H,
    x-matmuls on the opposite row-quadrant (X stored partition-swapped:
    v-half on partitions 0-63, u-half on 64-127).
  - DVE chain per group-step: STT (hn+bnh)*r, add xn, then three f16 2x-mode
    tensor ops for h' = n + z*(h-n).
"""

import os
import sys

sys.path.insert(0, "/opt/trn_rl_repo")

import numpy as np
from contextlib import ExitStack

HIDDEN = 64
OUT = 256
B = 8192
T = int(os.environ.get("GRU_T", 1024))
NCORES = 8
BC = B // NCORES          # 1024 batch per core
HB = BC // 2              # 512 batch per partition-half
UNROLL = 64               # timesteps per block
NFULL = T // UNROLL       # full blocks
TAIL = 0
NBLK = NFULL
NGROUP = 2                # phase-shifted batch groups per core
HG = HB // NGROUP         # 256 free-dim columns per group

_CACHE = {}


def _build():
    import concourse.bass as bass
    import concourse.tile as tile
    from concourse import bacc, mybir

    f16 = mybir.dt.float16
    f32 = mybir.dt.float32
    AF = mybir.ActivationFunctionType
    OP = mybir.AluOpType

    nc = bacc.Bacc("TRN2", target_bir_lowering=False, debug=False,
                   num_devices=NCORES)

    d_x = nc.dram_tensor("xt", [128, NBLK, HB], f16, kind="ExternalInput").ap()
    d_wr = nc.dram_tensor("wr", [128, 64], f16, kind="ExternalInput").ap()
    d_wz = nc.dram_tensor("wz", [128, 64], f16, kind="ExternalInput").ap()
    d_wn = nc.dram_tensor("wn", [128, 64], f16, kind="ExternalInput").ap()
    d_ohr = nc.dram_tensor("ohr", [128, UNROLL, 64], f16, kind="ExternalInput").ap()
    d_ohz = nc.dram_tensor("ohz", [128, UNROLL, 64], f16, kind="ExternalInput").ap()
    d_ohn = nc.dram_tensor("ohn", [128, UNROLL, 64], f16, kind="ExternalInput").ap()
    d_br = nc.dram_tensor("br", [128, 1], f32, kind="ExternalInput").ap()
    d_bz = nc.dram_tensor("bz", [128, 1], f32, kind="ExternalInput").ap()
    d_bnh = nc.dram_tensor("bnh", [128, 1], f32, kind="ExternalInput").ap()
    d_bni = nc.dram_tensor("bni", [128, 1], f32, kind="ExternalInput").ap()
    d_fcw = nc.dram_tensor("fcw", [128, OUT], f16, kind="ExternalInput").ap()
    d_fcb = nc.dram_tensor("fcb", [128, 2], f32, kind="ExternalInput").ap()
    d_out = nc.dram_tensor("out", [OUT, BC], f32, kind="ExternalOutput").ap()

    with tile.TileContext(nc) as tc, ExitStack() as ctx:
        singles = ctx.enter_context(tc.tile_pool(name="singles", bufs=1))
        work = ctx.enter_context(tc.tile_pool(name="work", bufs=4))
        psum = ctx.enter_context(tc.tile_pool(name="psum", bufs=1, space="PSUM"))

        X = singles.tile([128, NBLK, HB], f16)
        WR = singles.tile([128, 64], f16)
        WZ = singles.tile([128, 64], f16)
        WN = singles.tile([128, 64], f16)
        OHR = singles.tile([128, UNROLL, 64], f16)
        OHZ = singles.tile([128, UNROLL, 64], f16)
        OHN = singles.tile([128, UNROLL, 64], f16)
        BR = singles.tile([128, 1], f32)
        BZ = singles.tile([128, 1], f32)
        BNH = singles.tile([128, 1], f32)
        BNI = singles.tile([128, 1], f32)
        FCW = singles.tile([128, OUT], f16)
        FCB = singles.tile([128, 2], f32)
        H = singles.tile([128, HB], f16)

        for dst, src in ((X, d_x), (WR, d_wr), (WZ, d_wz), (WN, d_wn),
                         (OHR, d_ohr), (OHZ, d_ohz), (OHN, d_ohn),
                         (BR, d_br), (BZ, d_bz), (BNH, d_bnh), (BNI, d_bni),
                         (FCW, d_fcw), (FCB, d_fcb)):
            nc.gpsimd.dma_start(dst[:], src[:])
        nc.vector.memset(H[:], 0.0)

        # HAM warmup: ~20 back-to-back matmuls (>3.4us busy) lift the PE
        # clock gate to 8/8 once; the steady-state gaps are short enough to
        # keep it there.  Results are garbage and never read.
        warm = psum.tile([128, HG], f32, tag="PR0", name="warm")
        for _ in range(20):
            nc.tensor.matmul(warm[0:64, :], WR[0:64, :], H[0:64, 0:HG],
                             start=True, stop=True, tile_position=(0, 0))

        U = slice(0, 64)      # partitions: u-half of H / out, v-half of X
        V = slice(64, 128)    # partitions: v-half of H / out, u-half of X

        def mms_g(q, xsb, g, banks):
            # 12 matmuls for one group's step: gates r,z (h-part starts the
            # bank's accumulation group, x-part stops it), then NH / NX.
            mm = nc.tensor.matmul
            gsl = slice(g * HG, (g + 1) * HG)
            hu, hv = H[U, gsl], H[V, gsl]
            xu, xv = xsb[U, :, gsl], xsb[V, :, gsl]
            PR, PZ, PN, PX = banks
            for W, OH, P in ((WR, OHR, PR), (WZ, OHZ, PZ)):
                mm(P[V, :], W[V, :], hv, start=True, stop=False, tile_position=(64, 64))
                mm(P[V, :], OH[V, q, :], xv, start=False, stop=True, tile_position=(64, 64))
                mm(P[U, :], W[U, :], hu, start=True, stop=False, tile_position=(0, 0))
                mm(P[U, :], OH[U, q, :], xu, start=False, stop=True, tile_position=(0, 0))
            mm(PN[V, :], WN[V, :], hv, start=True, stop=True, tile_position=(64, 64))
            mm(PN[U, :], WN[U, :], hu, start=True, stop=True, tile_position=(0, 0))
            mm(PX[V, :], OHN[V, q, :], xv, start=True, stop=True, tile_position=(64, 64))
            mm(PX[U, :], OHN[U, q, :], xu, start=True, stop=True, tile_position=(0, 0))

        def sig_pair(g, banks):
            PR, PZ = banks[0], banks[1]
            SR = work.tile([128, HG], f16, tag=f"SR{g}", name="SR")
            SZ = work.tile([128, HG], f16, tag=f"SZ{g}", name="SZ")
            nc.scalar.activation(SR[:], PR[:], AF.Sigmoid, bias=BR[:])
            nc.scalar.activation(SZ[:], PZ[:], AF.Sigmoid, bias=BZ[:])
            return SR, SZ

        def stt_op(g, banks, SR):
            T1 = work.tile([128, HG], f16, tag=f"T1{g}", name="T1")
            nc.vector.scalar_tensor_tensor(T1[:], banks[2][:], BNH[:],
                                           SR[:], op0=OP.add, op1=OP.mult)
            return T1

        def t2_op(g, banks, T1):
            T2 = work.tile([128, HG], f16, tag=f"T2{g}", name="T2")
            nc.vector.tensor_add(T2[:], T1[:], banks[3][:])
            return T2

        def zb_op(g, SZ):
            # zb = 1 - z  (tensor_scalar, 4x mode)
            ZB = work.tile([128, HG], f16, tag=f"ZB{g}", name="ZB")
            nc.vector.tensor_scalar(ZB[:], SZ[:], -1.0, 1.0,
                                    op0=OP.mult, op1=OP.add)
            return ZB

        def p2_op(g, SZ):
            # p2 = z * h_{t-1}  (off the tanh critical path)
            P2 = work.tile([128, HG], f16, tag=f"P2{g}", name="P2")
            nc.vector.tensor_mul(P2[:], SZ[:], H[:, g * HG:(g + 1) * HG])
            return P2

        def tanh_op(g, T2):
            NN = work.tile([128, HG], f16, tag=f"NN{g}", name="NN")
            nc.scalar.activation(NN[:], T2[:], AF.Tanh, bias=BNI[:])
            return NN

        def p1_op(g, ZB, NN):
            P1 = work.tile([128, HG], f16, tag=f"P1{g}", name="P1")
            nc.vector.tensor_mul(P1[:], ZB[:], NN[:])
            return P1

        def add_op(g, P1, P2):
            # h' = (1-z)*n + z*h
            nc.vector.tensor_add(H[:, g * HG:(g + 1) * HG], P1[:], P2[:])

        def half_step(gA, pendA, gB, banksB):
            """Finish group gA's step (tanh, p1, h'-add) interleaved with
            group gB's first half-chain, so no DVE op directly follows the
            DVE op that produced its input."""
            if pendA is not None:
                ZBa, P2a, T2a = pendA
                NNa = tanh_op(gA, T2a)
                SRb, SZb = sig_pair(gB, banksB)
                P1a = p1_op(gA, ZBa, NNa)
                T1b = stt_op(gB, banksB, SRb)
                add_op(gA, P1a, P2a)
                T2b = t2_op(gB, banksB, T1b)
                ZBb = zb_op(gB, SZb)
                P2b = p2_op(gB, SZb)
            else:
                SRb, SZb = sig_pair(gB, banksB)
                T1b = stt_op(gB, banksB, SRb)
                T2b = t2_op(gB, banksB, T1b)
                ZBb = zb_op(gB, SZb)
                P2b = p2_op(gB, SZb)
            return (ZBb, P2b, T2b)

        def flush(g, pend):
            ZB, P2, T2 = pend
            NN = tanh_op(g, T2)
            P1 = p1_op(g, ZB, NN)
            add_op(g, P1, P2)

        def alloc_banks(g):
            PR = psum.tile([128, HG], f32, tag=f"PR{g}", name="PR")
            PZ = psum.tile([128, HG], f32, tag=f"PZ{g}", name="PZ")
            PN = psum.tile([128, HG], f32, tag=f"PN{g}", name="PN")
            PX = psum.tile([128, HG], f32, tag=f"PX{g}", name="PX")
            return (PR, PZ, PN, PX)

        # Software-pipelined: group 1 lags group 0 by half a step, so the
        # PE works on one group's matmuls while DVE/ACT run the other
        # group's elementwise chain.  The lag fills and drains inside each
        # block so the For_i body has no cross-iteration tile references.

        def body(blk, nstep):
            if isinstance(blk, int):
                blk = slice(blk, blk + 1)
            xsb = X[:, blk, :]
            pend1 = None
            for q in range(nstep):
                b0 = alloc_banks(0)
                mms_g(q, xsb, 0, b0)
                pend0 = half_step(1, pend1, 0, b0)
                b1 = alloc_banks(1)
                mms_g(q, xsb, 1, b1)
                pend1 = half_step(0, pend0, 1, b1)
            flush(1, pend1)

        if NFULL <= 1 or os.environ.get("GRU_NOHWLOOP"):
            for blk in range(NFULL):
                body(blk, UNROLL)
        else:
            with tc.For_i(0, NFULL, 1,
                          hint_engines=(mybir.EngineType.PE,)) as i:
                body(bass.ds(i, 1), UNROLL)

        # Final FC: out[o, b] = sum_k fc_w[o, k] h[b, k] + fc_b[o]
        for oh in range(2):
            osl = slice(oh * 128, (oh + 1) * 128)
            fc_u = psum.tile([128, HB], f32, tag="PR0")
            fc_v = psum.tile([128, HB], f32, tag="PR1")
            nc.tensor.matmul(fc_u[:], FCW[0:64, osl], H[0:64, :],
                             start=True, stop=True, tile_position=(0, 0))
            nc.tensor.matmul(fc_v[:], FCW[64:128, osl], H[64:128, :],
                             start=True, stop=True, tile_position=(64, 0))
            Ou = work.tile([128, HB], f32, tag="Ou")
            Ov = work.tile([128, HB], f32, tag="Ov")
            nc.scalar.activation(Ou[:], fc_u[:], AF.Identity,
                                 bias=FCB[:, oh:oh + 1])
            nc.scalar.activation(Ov[:], fc_v[:], AF.Identity,
                                 bias=FCB[:, oh:oh + 1])
            nc.gpsimd.dma_start(d_out[osl, 0:HB], Ou[:])
            nc.gpsimd.dma_start(d_out[osl, HB:BC], Ov[:])

    nc.compile()
    return nc


def _host_inputs(x, w_ih, w_hh, b_ih, b_hh, fc_w, fc_b):
    """Build the per-core in_maps (numpy, laid out exactly as SBUF tiles)."""
    f16 = np.float16
    f32 = np.float32
    x = np.asarray(x, f32)
    w_ih = np.asarray(w_ih, f32)
    w_hh = np.asarray(w_hh, f32)
    b_ih = np.asarray(b_ih, f32)
    b_hh = np.asarray(b_hh, f32)
    fc_w = np.asarray(fc_w, f32)
    fc_b = np.asarray(fc_b, f32)

    def oh(seg):
        w = w_ih[seg, 0]                            # [64]
        o = np.zeros((64, UNROLL, 64), f32)
        for q in range(UNROLL):
            o[q, q, :] = w
        return np.concatenate([o, o], 0).astype(f16)  # [128, UNROLL, 64]

    def wstack(seg):
        t = w_hh[seg, :].T                            # [64(k), 64(m)]
        return np.vstack([t, t]).astype(f16)

    def btile(v):
        return np.tile(v.reshape(-1, 1), (2, 1)).astype(f32)  # [128, 1]

    shared = {
        "wr": wstack(slice(0, 64)),
        "wz": wstack(slice(64, 128)),
        "wn": wstack(slice(128, 192)),
        "ohr": oh(slice(0, 64)),
        "ohz": oh(slice(64, 128)),
        "ohn": oh(slice(128, 192)),
        "br": btile(b_ih[0:64] + b_hh[0:64]),
        "bz": btile(b_ih[64:128] + b_hh[64:128]),
        "bnh": btile(b_hh[128:192]),
        "bni": btile(b_ih[128:192]),
        "fcw": np.vstack([fc_w.T, fc_w.T]).astype(f16),  # [128, 256]
        "fcb": np.stack([fc_b[0:128], fc_b[128:256]], 1).astype(f32),
    }

    in_maps = []
    for c in range(NCORES):
        xs = x[c * BC:(c + 1) * BC, :T, 0]            # [BC b, T t]
        xT = xs.T                                     # [T, BC]
        xr = xT.reshape(NBLK, UNROLL, BC)             # [blk, q, b]
        Xh = np.concatenate([xr[:, :, 0:HB].transpose(1, 0, 2),
                             xr[:, :, HB:BC].transpose(1, 0, 2)], 0)
        m = dict(shared)
        m["xt"] = np.ascontiguousarray(Xh).astype(f16)
        in_maps.append(m)
    return in_maps


def _run(in_maps, trace=False):
    from concourse import bass_utils
    if "nc" not in _CACHE:
        _CACHE["nc"] = _build()
    nc = _CACHE["nc"]
    res = bass_utils.run_bass_kernel_spmd(
        nc, in_maps, core_ids=list(range(NCORES)), trace=trace)
    return res


def kernel(**inputs):
    in_maps = _host_inputs(**inputs)
    res = _run(in_maps, trace=False)
    out = np.empty([B, OUT], np.float32)
    for c in range(NCORES):
        out[c * BC:(c + 1) * BC, :] = res.results[c]["out"].T
    return out

